# revision 1
# baseline (speedup 1.0000x reference)
"""EnhancedGovernanceAttention on 8 trn2 NeuronCores.

Sharding: tensor-parallel over heads with batch as secondary axis.
Core c handles batch b = c//4 and heads h in [4*(c%4), 4*(c%4)+4).
Each core computes a partial output [S, D] (its heads' contribution to
the out-projection); the host sums the partials and adds the bias.

Device kernel per core:
  - q,k projections computed transposed (qT[d, tok]) so QK^T needs no
    transpose; RoPE applied via a partition-shift SBUF DMA + DVE muls
    (cos/sin tables are host-provided, with the q tables pre-scaled by
    1/sqrt(dh) and the sin tables pre-signed for the rotate-half).
  - v computed in natural layout [tok, d].
  - scores per (head, 128-row tile of queries): one bf16 matmul per
    512-wide key tile (upper-triangle tiles skipped), bias added in
    natural layout: ACT computes log(1 + mem*gs + 1e-8), DVE folds
    prophetic*gs/2 + policy*gs + scores, gpsimd applies the causal
    fill on diagonal tiles via affine_select.
  - exp on ACT (bf16 out) with accum_out accumulating the softmax
    denominator for free.
  - P~ (bf16) transposed 128x128-blockwise via DMA XBAR transposes;
    AV matmul computes attn^T[d, tok] directly (lhsT = v tiles).
  - denominators transposed via a tiny PE transpose; reciprocal on DVE;
    normalization fused into the AV psum->sbuf copy.
  - out-projection over the core's 512 head-dims -> partial [S, D] f32.
"""

import sys

sys.path.insert(0, "/opt/trn_rl_repo")

import math

import ml_dtypes
import numpy as np

# problem shapes (hardcoded per contract)
B, S, D = 2, 1024, 2048
H, DH = 16, 128
GS = 0.1
ROPE_BASE = 10000.0
NCORES = 8
HPC = 4  # heads per core
DLOC = HPC * DH  # 512
TOK = S  # tokens per core (one batch each)
P = 128
KT = D // P  # 16 k-tiles over hidden dim
MASK_F32 = False  # stream governance masks as f32 (False: bf16, halves DMA)
EMIT_PIPE = False  # interleave per-head attention with projections
PP_BUFS = 3
QK_BUFS = 3
STAGE_BATCH = False
PTP_BUFS = 1

_CACHE = {}


def legalize_sync_waits(nc, max_waits=1):
    """This walrus build only encodes 1 sem wait per instruction; move
    overflow waits onto same-engine NoOps placed immediately before."""
    import concourse.mybir as mybir

    n_split = 0
    for f in nc.m.functions:
        for bb in f.blocks:
            il = bb.instructions
            i = 0
            while i < len(il):
                inst = il[i]
                si = inst.sync_info
                if si is not None and len(si.on_wait) > max_waits:
                    waits = list(si.on_wait)
                    keep = waits[:max_waits]
                    extra = waits[max_waits:]
                    pos = i
                    j = 0
                    while extra:
                        chunk, extra = extra[:max_waits], extra[max_waits:]
                        nop = mybir.InstNoOp(
                            name=f"{inst.name}-swx{j}",
                            engine=inst.engine,
                            bass_nofuse=True,
                            sync_info=mybir.SyncInfo(on_wait=chunk, on_update=[]),
                        )
                        il.insert(pos, nop)
                        pos += 1
                        j += 1
                    inst.sync_info = mybir.SyncInfo(
                        on_wait=keep, on_update=list(si.on_update)
                    )
                    i = pos + 1
                    n_split += 1
                else:
                    i += 1
    return n_split


def build_nc(repeat=1):
    import concourse.bass as bass
    import concourse.mybir as mybir
    from concourse.tile import TileContext

    f32 = mybir.dt.float32
    bf16 = mybir.dt.bfloat16
    mask_dt = f32 if MASK_F32 else bf16
    Alu = mybir.AluOpType
    Act = mybir.ActivationFunctionType

    nc = bass.Bass()

    xt = nc.dram_tensor("xt", [D, TOK], bf16, kind="ExternalInput")
    wq = nc.dram_tensor("wq", [D, DLOC], bf16, kind="ExternalInput")
    wk = nc.dram_tensor("wk", [D, DLOC], bf16, kind="ExternalInput")
    wv = nc.dram_tensor("wv", [D, DLOC], bf16, kind="ExternalInput")
    wo = nc.dram_tensor("wo", [DLOC, D], bf16, kind="ExternalInput")
    pm = nc.dram_tensor("pm", [HPC, S, S], mask_dt, kind="ExternalInput")
    pol = nc.dram_tensor("pol", [HPC, S, S], mask_dt, kind="ExternalInput")
    mem = nc.dram_tensor("mem", [HPC, S, S], mask_dt, kind="ExternalInput")
    cosq = nc.dram_tensor("cosq", [P, TOK], f32, kind="ExternalInput")
    sinq = nc.dram_tensor("sinq", [P, TOK], f32, kind="ExternalInput")
    cosk = nc.dram_tensor("cosk", [P, TOK], f32, kind="ExternalInput")
    sink = nc.dram_tensor("sink", [P, TOK], f32, kind="ExternalInput")
    ident3 = nc.dram_tensor("ident3", [P, 3 * P], bf16, kind="ExternalInput")
    cneg = nc.dram_tensor("cneg", [4, P, 512], bf16, kind="ExternalInput")
    out = nc.dram_tensor("out", [TOK, D], bf16, kind="ExternalOutput")

    xt_t = xt.rearrange("(kt p) t -> p kt t", p=P)
    wq_t = wq.rearrange("(kt p) c -> p kt c", p=P)
    wk_t = wk.rearrange("(kt p) c -> p kt c", p=P)
    wv_t = wv.rearrange("(kt p) c -> p kt c", p=P)
    wo_t = wo.rearrange("(kt p) c -> p kt c", p=P)

    with TileContext(nc) as tc:
        with (
            tc.tile_pool(name="persist", bufs=1) as persist,
            tc.tile_pool(name="ppsum", bufs=PP_BUFS, space="PSUM") as ppsum,
            tc.tile_pool(name="qk_psum", bufs=QK_BUFS, space="PSUM") as qk_psum,
            tc.tile_pool(name="av_psum", bufs=2, space="PSUM") as av_psum,
            tc.tile_pool(name="work", bufs=3) as work,
            tc.tile_pool(name="masks", bufs=2) as masks_p,
            tc.tile_pool(name="pnatp", bufs=3) as pnatp,
            tc.tile_pool(name="ptp", bufs=PTP_BUFS) as ptp,
            tc.tile_pool(name="lp", bufs=2) as lp,
            tc.tile_pool(name="logmp", bufs=8) as logmp,
        ):
            # ---- persistent tiles (repeat>1 is a timing-only mode that
            # runs the whole body multiple times in one NEFF)
            for _rep in range(repeat):
                body(nc, tc, locals())
    legalize_sync_waits(nc, max_waits=1)
    return nc


def body(nc, tc, env):
    import concourse.mybir as mybir

    f32 = mybir.dt.float32
    bf16 = mybir.dt.bfloat16
    mask_dt = f32 if MASK_F32 else bf16
    Alu = mybir.AluOpType
    Act = mybir.ActivationFunctionType
    persist = env["persist"]
    ppsum = env["ppsum"]
    qk_psum = env["qk_psum"]
    av_psum = env["av_psum"]
    work = env["work"]
    maskp = env["masks_p"]
    pnatp = env["pnatp"]
    ptp = env["ptp"]
    lp = env["lp"]
    logmp = env["logmp"]
    xt_t, wq_t, wk_t, wv_t, wo_t = (
        env["xt_t"],
        env["wq_t"],
        env["wk_t"],
        env["wv_t"],
        env["wo_t"],
    )
    cosq, sinq, cosk, sink = env["cosq"], env["sinq"], env["cosk"], env["sink"]
    pm, pol, mem, out = env["pm"], env["pol"], env["mem"], env["out"]
    ident3, cneg = env["ident3"], env["cneg"]

    # ---- persistent tiles
    qT = persist.tile([P, HPC, TOK], bf16)  # [d, h, tok]
    kT = persist.tile([P, HPC, TOK], bf16)
    v_sb = persist.tile([P, TOK // P, DLOC], bf16)  # [tokp, tokt, hd]
    attnT = persist.tile([P, HPC, TOK], bf16)  # [d, h(=ktile), tok]
    lnbias = persist.tile([P, 1], f32)
    nc.vector.memset(lnbias[:], 1.0 + 1e-8)
    id3_sb = persist.tile([P, 3 * P], bf16)
    nc.sync.dma_start(id3_sb[:], ident3[:])
    cneg_sb = persist.tile([P, 4, 512], bf16)
    nc.sync.dma_start(cneg_sb[:], cneg.rearrange("j p c -> p j c"))

    pTs = {}

    def qk_phase(h):
        """scores + bias (PE psum accumulation) + exp + normalize +
        transpose for one head."""
        pT = ptp.tile([P, TOK // P, TOK], bf16, tag="pt")
        pTs[h] = pT
        l_all = lp.tile([P, TOK // P], f32, tag="l_all")
        for mig in range(TOK // P // 2):  # pairs of query row-tiles
            n_sk = (mig // 2 + 1) * 512
            rsl = slice(mig * 2 * P, (mig + 1) * 2 * P)
            mem_t = maskp.tile([P, 2, TOK], mask_dt, tag="memt")
            nc.sync.dma_start(
                mem_t[:, :, :n_sk],
                mem[h, rsl, :n_sk].rearrange("(j p) c -> p j c", p=P),
            )
            pm_t = maskp.tile([P, 2, TOK], mask_dt, tag="pmt")
            nc.sync.dma_start(
                pm_t[:, :, :n_sk],
                pm[h, rsl, :n_sk].rearrange("(j p) c -> p j c", p=P),
            )
            pol_t = maskp.tile([P, 2, TOK], mask_dt, tag="polt")
            nc.scalar.dma_start(
                pol_t[:, :, :n_sk],
                pol[h, rsl, :n_sk].rearrange("(j p) c -> p j c", p=P),
            )
            # batch the Ln ops for this row-pair ahead of the exps so the
            # ACT engine switches activation tables once, not per tile
            lns = {}
            for j in range(2):
                for ni in range(n_sk // 512):
                    ksl = slice(ni * 512, (ni + 1) * 512)
                    logm = logmp.tile([P, 512], f32, tag="logm")
                    nc.scalar.activation(
                        logm[:],
                        mem_t[:, j, ksl],
                        Act.Ln,
                        bias=lnbias[:],
                        scale=GS,
                    )
                    lns[(j, ni)] = logm
            for j in range(2):
                mi = mig * 2 + j
                pnat = pnatp.tile([P, TOK], bf16, tag="pnat")
                for ni in range(n_sk // 512):
                    ksl = slice(ni * 512, (ni + 1) * 512)
                    s1 = work.tile([P, 512], f32, tag="s1")
                    nc.vector.scalar_tensor_tensor(
                        s1[:],
                        pm_t[:, j, ksl],
                        GS * 0.5,
                        lns[(j, ni)][:],
                        Alu.mult,
                        Alu.add,
                    )
                    s2 = work.tile([P, 512], f32, tag="s2")
                    nc.vector.scalar_tensor_tensor(
                        s2[:],
                        pol_t[:, j, ksl],
                        GS,
                        s1[:],
                        Alu.mult,
                        Alu.add,
                    )
                    if ni == mi // 4:  # diagonal: add causal -1e30 pattern
                        nc.gpsimd.tensor_add(s2[:], s2[:], cneg_sb[:, mi % 4, :])
                    ps = qk_psum.tile([P, 512], f32, tag="qk")
                    nc.tensor.matmul(
                        ps[:],
                        qT[:, h, mi * P : (mi + 1) * P],
                        kT[:, h, ksl],
                        start=True,
                        stop=True,
                    )
                    s3 = work.tile([P, 512], f32, tag="s3")
                    nc.vector.tensor_tensor(s3[:], s2[:], ps[:], Alu.add)
                    if ni == 0:
                        acc = l_all[:, mi : mi + 1]
                    else:
                        ltmp = lp.tile([P, 1], f32, tag="ltmp")
                        acc = ltmp[:]
                    nc.scalar.activation(
                        pnat[:, ksl],
                        s3[:],
                        Act.Exp,
                        accum_out=acc,
                    )
                    if ni == 1:
                        nc.vector.tensor_add(
                            l_all[:, mi : mi + 1],
                            l_all[:, mi : mi + 1],
                            ltmp[:],
                        )
                # normalize the row (per-partition 1/l), then one merged
                # XBAR transpose into pT's column block
                rec = lp.tile([P, 1], f32, tag="rec")
                nc.vector.reciprocal(rec[:], l_all[:, mi : mi + 1])
                nc.gpsimd.tensor_scalar_mul(
                    pnat[:, :n_sk], pnat[:, :n_sk], rec[:]
                )
                nc.scalar.dma_start_transpose(
                    pT[:, 0 : n_sk // P, mi * P : (mi + 1) * P],
                    pnat[:, :n_sk],
                )

    def av_phase(h):
        pT = pTs.pop(h)
        for nj in range(2):
            n_ki = 4 if nj == 0 else 8
            ps = av_psum.tile([P, 512], f32, tag="av")
            for ki in range(n_ki):
                nc.tensor.matmul(
                    ps[:],
                    v_sb[:, ki, h * P : (h + 1) * P],
                    pT[:, ki, nj * 512 : (nj + 1) * 512],
                    start=(ki == 0),
                    stop=(ki == n_ki - 1),
                )
            nc.scalar.copy(attnT[:, h, nj * 512 : (nj + 1) * 512], ps[:])

    # ---- projections + attention, software-pipelined across heads
    with tc.tile_pool(name="proj", bufs=1) as proj, tc.tile_pool(
        name="wstream", bufs=2
    ) as wstream:
        xt_sb = proj.tile([P, KT, TOK], bf16)
        nc.sync.dma_start(xt_sb[:, :, 0:512], xt_t[:, :, 0:512])
        nc.sync.dma_start(xt_sb[:, :, 512:TOK], xt_t[:, :, 512:TOK])
        tabs = {}
        for name, tab in (
            ("cosq", cosq),
            ("sinq", sinq),
            ("cosk", cosk),
            ("sink", sink),
        ):
            t = proj.tile([P, TOK], bf16, tag=name)
            nc.gpsimd.dma_start(t[:], tab[:])  # casts f32 -> bf16
            tabs[name] = t

        def proj_qk(which, m):
            wt = wq_t if which == "q" else wk_t
            dest = qT if which == "q" else kT
            cos_t = tabs["cosq" if which == "q" else "cosk"]
            sin_t = tabs["sinq" if which == "q" else "sink"]
            w_cur = wstream.tile([P, KT, P], bf16, tag="wqk")
            nc.sync.dma_start(w_cur[:], wt[:, :, m * P : (m + 1) * P])
            for n in range(TOK // 512):
                tsl = slice(n * 512, (n + 1) * 512)
                ps = ppsum.tile([P, 512], f32, tag="pp")
                for k in range(KT):
                    nc.tensor.matmul(
                        ps[:],
                        w_cur[:, k, :],
                        xt_sb[:, k, tsl],
                        start=(k == 0),
                        stop=(k == KT - 1),
                    )
                raw = work.tile([P, 512], f32, tag="rope_raw")
                nc.scalar.copy(raw[:], ps[:])
                swp = work.tile([P, 512], f32, tag="rope_swp")
                nc.sync.dma_start(swp[0:64, :], raw[64:128, :])
                nc.sync.dma_start(swp[64:128, :], raw[0:64, :])
                nc.vector.tensor_tensor(raw[:], ps[:], cos_t[:, tsl], Alu.mult)
                nc.vector.tensor_tensor(swp[:], swp[:], sin_t[:, tsl], Alu.mult)
                nc.vector.tensor_tensor(dest[:, m, tsl], raw[:], swp[:], Alu.add)

        def proj_v():
            wv_sb = proj.tile([P, KT, DLOC], bf16, tag="wv")
            nc.sync.dma_start(wv_sb[:], wv_t)
            for mt in range(TOK // P):
                ps = ppsum.tile([P, DLOC], f32, tag="pp")
                for k in range(KT):
                    nc.tensor.matmul(
                        ps[:],
                        xt_sb[:, k, mt * P : (mt + 1) * P],
                        wv_sb[:, k, :],
                        start=(k == 0),
                        stop=(k == KT - 1),
                    )
                nc.scalar.copy(v_sb[:, mt, :], ps[:])

        if EMIT_PIPE:
            proj_qk("q", 0)
            proj_qk("k", 0)
            proj_v()
            qk_phase(0)
            proj_qk("q", 1)
            proj_qk("k", 1)
            qk_phase(1)
            av_phase(0)
            proj_qk("q", 2)
            proj_qk("k", 2)
            qk_phase(2)
            av_phase(1)
            proj_qk("q", 3)
            proj_qk("k", 3)
            qk_phase(3)
            av_phase(2)
            av_phase(3)
        else:
            proj_v()
            for m in range(HPC):
                proj_qk("q", m)
                proj_qk("k", m)
            for h in range(HPC):
                qk_phase(h)
                av_phase(h)

    # ---- out projection partial (wo streamed per n-chunk)
    with tc.tile_pool(name="outp", bufs=3) as outp, tc.tile_pool(
        name="wop", bufs=2
    ) as wop:
        for n in range(D // 512):
            wo_sb = wop.tile([P, HPC, 512], bf16, tag="wo")
            nc.sync.dma_start(wo_sb[:], wo_t[:, :, n * 512 : (n + 1) * 512])
            for mt in range(TOK // P):
                ps = ppsum.tile([P, 512], f32, tag="pp")
                for kt in range(HPC):
                    nc.tensor.matmul(
                        ps[:],
                        attnT[:, kt, mt * P : (mt + 1) * P],
                        wo_sb[:, kt, :],
                        start=(kt == 0),
                        stop=(kt == HPC - 1),
                    )
                ot = outp.tile([P, 512], bf16, tag="ot")
                nc.scalar.copy(ot[:], ps[:])
                nc.sync.dma_start(
                    out[mt * P : (mt + 1) * P, n * 512 : (n + 1) * 512],
                    ot[:],
                )


def _rope_tables():
    """cos/sin tables in transposed-projection layout [128 dims, TOK],
    with rotate-half sign folded into sin and 1/sqrt(dh) folded into the
    q tables."""
    inv_freq = 1.0 / (
        ROPE_BASE ** (np.arange(0, DH, 2, dtype=np.float32) / DH)
    )  # [64]
    t = np.arange(S, dtype=np.float32)
    freqs = np.outer(t, inv_freq)  # [S, 64]
    cos = np.cos(freqs)
    sin = np.sin(freqs)
    cos2 = np.empty((P, TOK), np.float32)
    sin2 = np.empty((P, TOK), np.float32)
    cos2[0:64] = cos.T
    cos2[64:128] = cos.T
    sin2[0:64] = -sin.T
    sin2[64:128] = sin.T
    scale = 1.0 / math.sqrt(DH)
    return cos2 * scale, sin2 * scale, cos2, sin2


def make_in_maps(x, prophetic_mask, policy_mask, memory_weights, Wq, Wk, Wv, Wo):
    from concurrent.futures import ThreadPoolExecutor

    bf16 = ml_dtypes.bfloat16
    mask_np_dt = np.float32 if MASK_F32 else bf16
    cosq, sinq, cosk, sink = _rope_tables()
    eye = np.eye(P, dtype=np.float32)
    ident3 = np.concatenate([eye * (GS * 0.5), eye * GS, eye], 1).astype(bf16)
    jj, pp, cc = np.meshgrid(
        np.arange(4), np.arange(P), np.arange(512), indexing="ij"
    )
    cneg = np.where(cc <= 128 * jj + pp, 0.0, -1.0e30).astype(bf16)
    wq_b = np.ascontiguousarray(Wq).astype(bf16)
    wk_b = np.ascontiguousarray(Wk).astype(bf16)
    wv_b = np.ascontiguousarray(Wv).astype(bf16)
    wo_b = np.ascontiguousarray(Wo).astype(bf16)

    def shard(c):
        b = c // 4
        g = c % 4
        cols = slice(DLOC * g, DLOC * (g + 1))
        hsl = slice(HPC * g, HPC * (g + 1))
        return {
            "xt": np.ascontiguousarray(x[b].T).astype(bf16),
            "wq": np.ascontiguousarray(wq_b[:, cols]),
            "wk": np.ascontiguousarray(wk_b[:, cols]),
            "wv": np.ascontiguousarray(wv_b[:, cols]),
            "wo": np.ascontiguousarray(wo_b[cols, :]),
            "pm": prophetic_mask[b, hsl].astype(mask_np_dt),
            "pol": policy_mask[b, hsl].astype(mask_np_dt),
            "mem": memory_weights[b, hsl].astype(mask_np_dt),
            "cosq": cosq,
            "sinq": sinq,
            "cosk": cosk,
            "sink": sink,
            "ident3": ident3,
            "cneg": cneg,
        }

    with ThreadPoolExecutor(8) as ex:
        in_maps = list(ex.map(shard, range(NCORES)))
    return in_maps


def kernel(x, prophetic_mask, policy_mask, memory_weights, Wq, Wk, Wv, Wo, bo):
    from concourse.bass_utils import run_bass_kernel_spmd

    if "nc" not in _CACHE:
        _CACHE["nc"] = build_nc()
    nc = _CACHE["nc"]
    in_maps = make_in_maps(
        x, prophetic_mask, policy_mask, memory_weights, Wq, Wk, Wv, Wo
    )
    res = run_bass_kernel_spmd(nc, in_maps, list(range(NCORES)))
    out = np.zeros((B, S, D), np.float32)
    for c in range(NCORES):
        out[c // 4] += res.results[c]["out"].astype(np.float32)
    out += np.asarray(bo, np.float32)[None, None, :]
    return out



# revision 13
# speedup vs baseline: 2.7646x; 2.7646x over previous
"""EnhancedGovernanceAttention on 8 trn2 NeuronCores.

Sharding: tensor-parallel over heads with batch as secondary axis.
Core c handles batch b = c//4 and heads h in [4*(c%4), 4*(c%4)+4).
Each core computes a partial output [S, D] (its heads' contribution to
the out-projection); the host sums the partials and adds the bias.

Device kernel per core (v2):
  - q,k projections computed transposed (qT[d, tok]); RoPE via a
    partition-shift SWDGE copy + all-bf16 DVE muls (tables host-built,
    q tables pre-scaled by 1/sqrt(dh), sin pre-signed).
  - governance bias never touches ACT/DVE beyond one bf16 add:
    host pre-scales pm*=GS/2, pol*=GS, mem*=log1p(GS) (chord fit of
    log(1+GS*m), max logit spread ~6e-4); device computes
    s = pm+pol (DVE bf16), then PE accumulates id@s + id@mem (+ id@cneg
    on diagonal tiles) straight into the QK^T psum.
  - causal exact-trim at 128 cols: row-block mi only computes keys
    <= (mi+1)*128 (qk/bias matmuls, exp, mask DMA all shrink ~25%).
  - exp on ACT reads PSUM directly (bf16 out, accum_out denominators);
    only activation table in the whole kernel is Exp.
  - pnat laid out [p, kj, mi, c] so ONE merged XBAR transpose per head
    produces pT[key, kj*8+mi, q] tiles contiguously.
  - AV exact per query-block: psum[dh,128q] over ki<=mi; Pool copies to
    attnT (f32->bf16).
  - out-projection partials DMA'd PSUM->DRAM directly (f32->bf16).
"""

import sys

sys.path.insert(0, "/opt/trn_rl_repo")

import math

import ml_dtypes
import numpy as np

# problem shapes (hardcoded per contract)
B, S, D = 2, 1024, 2048
H, DH = 16, 128
GS = 0.1
ROPE_BASE = 10000.0
NCORES = 8
HPC = 4  # heads per core
DLOC = HPC * DH  # 512
TOK = S  # tokens per core (one batch each)
P = 128
KT = D // P  # 16 k-tiles over hidden dim
NMI = TOK // P  # 8 query row-blocks per head

_CACHE = {}


def legalize_sync_waits(nc, max_waits=1):
    """This walrus build only encodes 1 sem wait per instruction; move
    overflow waits onto same-engine NoOps placed immediately before."""
    import concourse.mybir as mybir

    n_split = 0
    for f in nc.m.functions:
        for bb in f.blocks:
            il = bb.instructions
            i = 0
            while i < len(il):
                inst = il[i]
                si = inst.sync_info
                if si is not None and len(si.on_wait) > max_waits:
                    waits = list(si.on_wait)
                    keep = waits[:max_waits]
                    extra = waits[max_waits:]
                    pos = i
                    j = 0
                    while extra:
                        chunk, extra = extra[:max_waits], extra[max_waits:]
                        nop = mybir.InstNoOp(
                            name=f"{inst.name}-swx{j}",
                            engine=inst.engine,
                            bass_nofuse=True,
                            sync_info=mybir.SyncInfo(on_wait=chunk, on_update=[]),
                        )
                        il.insert(pos, nop)
                        pos += 1
                        j += 1
                    inst.sync_info = mybir.SyncInfo(
                        on_wait=keep, on_update=list(si.on_update)
                    )
                    i = pos + 1
                    n_split += 1
                else:
                    i += 1
    return n_split


def build_nc(repeat=1):
    import concourse.bass as bass
    import concourse.mybir as mybir
    from concourse.tile import TileContext

    f32 = mybir.dt.float32
    bf16 = mybir.dt.bfloat16

    nc = bass.Bass()

    xt = nc.dram_tensor("xt", [D, TOK], bf16, kind="ExternalInput")
    wq = nc.dram_tensor("wq", [D, DLOC], bf16, kind="ExternalInput")
    wk = nc.dram_tensor("wk", [D, DLOC], bf16, kind="ExternalInput")
    wv = nc.dram_tensor("wv", [D, DLOC], bf16, kind="ExternalInput")
    wo = nc.dram_tensor("wo", [DLOC, D], bf16, kind="ExternalInput")
    pm = nc.dram_tensor("pm", [HPC, S, S], bf16, kind="ExternalInput")
    pol = nc.dram_tensor("pol", [HPC, S, S], bf16, kind="ExternalInput")
    mem = nc.dram_tensor("mem", [HPC, S, S], bf16, kind="ExternalInput")
    cosq = nc.dram_tensor("cosq", [P, TOK], f32, kind="ExternalInput")
    sinq = nc.dram_tensor("sinq", [P, TOK], f32, kind="ExternalInput")
    cosk = nc.dram_tensor("cosk", [P, TOK], f32, kind="ExternalInput")
    sink = nc.dram_tensor("sink", [P, TOK], f32, kind="ExternalInput")
    ident = nc.dram_tensor("ident", [P, P], bf16, kind="ExternalInput")
    cneg = nc.dram_tensor("cneg", [P, P], bf16, kind="ExternalInput")
    out = nc.dram_tensor("out", [TOK, D], bf16, kind="ExternalOutput")

    env = dict(
        xt_t=xt.rearrange("(kt p) t -> p kt t", p=P),
        wq_t=wq.rearrange("(kt p) c -> p kt c", p=P),
        wk_t=wk.rearrange("(kt p) c -> p kt c", p=P),
        wv_t=wv.rearrange("(kt p) c -> p kt c", p=P),
        wo_t=wo.rearrange("(kt p) c -> p kt c", p=P),
        pm=pm, pol=pol, mem=mem, out=out,
        cosq=cosq, sinq=sinq, cosk=cosk, sink=sink,
        ident=ident, cneg=cneg,
    )

    with TileContext(nc) as tc:
        with (
            tc.tile_pool(name="persist", bufs=1) as persist,
            tc.tile_pool(name="ppsum", bufs=2, space="PSUM") as ppsum,
            tc.tile_pool(name="qk_psum", bufs=4, space="PSUM") as qk_psum,
            tc.tile_pool(name="av_psum", bufs=2, space="PSUM") as av_psum,
            tc.tile_pool(name="work", bufs=2) as work,
            tc.tile_pool(name="outp", bufs=3) as outp,
            tc.tile_pool(name="masks", bufs=3) as maskp,
            tc.tile_pool(name="srect", bufs=2) as srectp,
            tc.tile_pool(name="pnatp", bufs=1) as pnatp,
            tc.tile_pool(name="ptp", bufs=1) as ptp,
            tc.tile_pool(name="lp", bufs=2) as lp,
            tc.tile_pool(name="wstream", bufs=2) as wstream,
            tc.tile_pool(name="wvp", bufs=1) as wvp,
        ):
            env.update(
                persist=persist, ppsum=ppsum, qk_psum=qk_psum,
                av_psum=av_psum, work=work, outp=outp, maskp=maskp,
                srectp=srectp, pnatp=pnatp, ptp=ptp, lp=lp,
                wstream=wstream, wvp=wvp,
            )
            for _rep in range(repeat):
                body(nc, tc, env)
    legalize_sync_waits(nc, max_waits=1)
    return nc


def body(nc, tc, env):
    import concourse.mybir as mybir

    f32 = mybir.dt.float32
    bf16 = mybir.dt.bfloat16
    Alu = mybir.AluOpType
    Act = mybir.ActivationFunctionType

    persist = env["persist"]
    ppsum = env["ppsum"]
    qk_psum = env["qk_psum"]
    av_psum = env["av_psum"]
    work = env["work"]
    outp = env["outp"]
    maskp = env["maskp"]
    srectp = env["srectp"]
    pnatp = env["pnatp"]
    ptp = env["ptp"]
    lp = env["lp"]
    wstream = env["wstream"]
    wvp = env["wvp"]
    xt_t, wq_t, wk_t, wv_t, wo_t = (
        env["xt_t"], env["wq_t"], env["wk_t"], env["wv_t"], env["wo_t"],
    )
    pm, pol, mem, out = env["pm"], env["pol"], env["mem"], env["out"]
    ident, cneg = env["ident"], env["cneg"]

    # ---- persistent tiles
    qT = persist.tile([P, HPC, TOK], bf16)  # [dh, h, tok]
    kT = persist.tile([P, HPC, TOK], bf16)
    v_sb = persist.tile([P, NMI, DLOC], bf16)  # [tokp, tokt, hd]
    attnT = persist.tile([P, HPC, TOK], bf16)  # [dh, h, tok]
    id_sb = persist.tile([P, P], bf16)
    nc.scalar.dma_start(id_sb[:], ident[:])
    cneg_sb = persist.tile([P, P], bf16)
    nc.scalar.dma_start(cneg_sb[:], cneg[:])
    xt_lo_a = persist.tile([P, KT // 2, 512], bf16)
    nc.sync.dma_start(xt_lo_a[:], xt_t[:, 0 : KT // 2, 0:512])
    xt_lo_b = persist.tile([P, KT // 2, 512], bf16)
    xt_hi_a = persist.tile([P, KT // 2, 512], bf16)
    xt_hi_b = persist.tile([P, KT // 2, 512], bf16)

    def xt_part(k, hi):
        if k < KT // 2:
            t = xt_hi_a if hi else xt_lo_a
            return t[:, k, :]
        t = xt_hi_b if hi else xt_lo_b
        return t[:, k - KT // 2, :]
    tabs = {}
    for name in ("cosq", "sinq", "cosk", "sink"):
        t = persist.tile([P, TOK], bf16, tag=name)
        nc.gpsimd.dma_start(t[:], env[name][:])  # casts f32 -> bf16
        tabs[name] = t

    # mask sets stream per (head, row-pair): three [P, 2, cols] tiles,
    # causal-trimmed to cols = (2p+2)*128; prefetched 2 sets ahead.
    mask_order = [(h, p) for h in range(HPC) for p in range(NMI // 2)]
    mask_tiles = {}
    mask_next = [0]

    def mask_prefetch():
        if mask_next[0] >= len(mask_order):
            return
        h, p = mask_order[mask_next[0]]
        mask_next[0] += 1
        cols = (2 * p + 2) * P
        rsl = slice(2 * p * P, (2 * p + 2) * P)
        tl = {}
        for name, dram, eng in (
            ("pm", pm, nc.sync), ("pol", pol, nc.scalar), ("mem", mem, nc.gpsimd)
        ):
            t = maskp.tile([P, 2, TOK], bf16, tag=name)
            eng.dma_start(
                t[:, :, :cols],
                dram[h, rsl, :cols].rearrange("(j p) c -> p j c", p=P),
            )
            tl[name] = t
        mask_tiles[(h, p)] = tl

    pTs = {}

    def qk_phase(h):
        # pnat[p, mi, k] row-major; per-row exact-trim XBAR transposes
        # land in pT[p_key, kj, mi, c_q].
        pnat = pnatp.tile([P, NMI, TOK], bf16, tag="pnat")
        pT = ptp.tile([P, NMI, NMI, P], bf16, tag="pt")
        pTs[h] = pT
        l_all = lp.tile([P, NMI], f32, tag="l_all")
        rec = lp.tile([P, NMI], f32, tag="rec")

        for p in range(NMI // 2):
            mask_prefetch()
            tl = mask_tiles.pop((h, p))
            cols_pair = (2 * p + 2) * P
            # s = pm + pol (host already folded GS/2, GS scales)
            s_t = srectp.tile([P, 2, TOK], bf16, tag="s")
            nc.vector.tensor_tensor(
                s_t[:, :, :cols_pair],
                tl["pm"][:, :, :cols_pair],
                tl["pol"][:, :, :cols_pair],
                Alu.add,
            )
            mem_t = tl["mem"]
            for jr in range(2):
                mi = 2 * p + jr
                ncols = (mi + 1) * P
                for ti in range((ncols + 511) // 512):
                    k0 = ti * 512
                    j = min(4, (ncols - k0) // P)  # 128-col blocks here
                    ps = qk_psum.tile([P, 4, P], f32, tag="qk")
                    nc.tensor.matmul(
                        ps[:, 0:j, :],
                        qT[:, h, mi * P : (mi + 1) * P],
                        kT[:, h, k0 : k0 + j * P],
                        start=True,
                        stop=False,
                    )
                    nc.tensor.matmul(
                        ps[:, 0:j, :],
                        id_sb[:],
                        s_t[:, jr, k0 : k0 + j * P],
                        start=False,
                        stop=False,
                    )
                    is_last_tile = k0 + j * P == ncols
                    nc.tensor.matmul(
                        ps[:, 0:j, :],
                        id_sb[:],
                        mem_t[:, jr, k0 : k0 + j * P],
                        start=False,
                        stop=not is_last_tile,
                    )
                    if is_last_tile:  # diagonal block: causal -1e30 fill
                        nc.tensor.matmul(
                            ps[:, j - 1, :], id_sb[:], cneg_sb[:],
                            start=False, stop=True,
                        )
                    # exp straight out of PSUM; accumulate denominator
                    if ti == 0:
                        acc = l_all[:, mi : mi + 1]
                    else:
                        ltmp = lp.tile([P, 1], f32, tag="ltmp")
                        acc = ltmp[:]
                    nc.scalar.activation(
                        pnat[:, mi, k0 : k0 + j * P],
                        ps[:, 0:j, :],
                        Act.Exp,
                        accum_out=acc,
                    )
                    if ti == 1:
                        nc.vector.tensor_add(
                            l_all[:, mi : mi + 1], l_all[:, mi : mi + 1], ltmp[:]
                        )
                nc.vector.reciprocal(rec[:, mi : mi + 1], l_all[:, mi : mi + 1])
                nc.vector.tensor_scalar_mul(
                    pnat[:, mi, :ncols],
                    pnat[:, mi, :ncols],
                    rec[:, mi : mi + 1],
                )
                nc.sync.dma_start_transpose(
                    pT[:, 0 : mi + 1, mi, :], pnat[:, mi, :ncols]
                )

    def av_phase(h):
        pT = pTs.pop(h)
        for mi in range(NMI):
            ps = av_psum.tile([P, P], f32, tag="av")
            for ki in range(mi + 1):
                nc.tensor.matmul(
                    ps[:],
                    v_sb[:, ki, h * P : (h + 1) * P],
                    pT[:, ki, mi, :],
                    start=(ki == 0),
                    stop=(ki == mi),
                )
            nc.vector.tensor_scalar_mul(
                attnT[:, h, mi * P : (mi + 1) * P], ps[:], 1.0
            )

    def proj_qk(which, m, xt_hi_dma=None):
        wt = wq_t if which == "q" else wk_t
        dest = qT if which == "q" else kT
        cos_t = tabs["cosq" if which == "q" else "cosk"]
        sin_t = tabs["sinq" if which == "q" else "sink"]
        w_cur = wstream.tile([P, KT, P], bf16, tag="wqk")
        nc.sync.dma_start(w_cur[:], wt[:, :, m * P : (m + 1) * P])
        if xt_hi_dma is not None:  # slot the rest of x behind q0's weights
            lo_b, hi_a, hi_b, dram = xt_hi_dma
            nc.sync.dma_start(lo_b[:], dram[:, KT // 2 : KT, 0:512])
            nc.sync.dma_start(hi_a[:], dram[:, 0 : KT // 2, 512:TOK])
            nc.sync.dma_start(hi_b[:], dram[:, KT // 2 : KT, 512:TOK])
        raw = work.tile([P, TOK], bf16, tag="raw")
        for n in range(TOK // 512):
            ps = ppsum.tile([P, 512], f32, tag="pp")
            for k in range(KT):
                nc.tensor.matmul(
                    ps[:],
                    w_cur[:, k, :],
                    xt_part(k, n == 1),
                    start=(k == 0),
                    stop=(k == KT - 1),
                )
            nc.scalar.copy(raw[:, n * 512 : (n + 1) * 512], ps[:])
        swp = work.tile([P, TOK], bf16, tag="swp")
        nc.gpsimd.dma_start(swp[0:64, :], raw[64:128, :])
        nc.gpsimd.dma_start(swp[64:128, :], raw[0:64, :])
        nc.vector.tensor_tensor(dest[:, m, :], raw[:], cos_t[:], Alu.mult)
        nc.vector.tensor_tensor(swp[:], swp[:], sin_t[:], Alu.mult)
        nc.vector.tensor_add(dest[:, m, :], dest[:, m, :], swp[:])

    def proj_v():
        wv_sb = wvp.tile([P, KT, DLOC], bf16, tag="wv")
        nc.sync.dma_start(wv_sb[:], wv_t)
        for mt in range(NMI):
            ps = ppsum.tile([P, DLOC], f32, tag="pp")
            for k in range(KT):
                xp = xt_part(k, mt >= 4)
                nc.tensor.matmul(
                    ps[:],
                    xp[:, (mt % 4) * P : (mt % 4 + 1) * P],
                    wv_sb[:, k, :],
                    start=(k == 0),
                    stop=(k == KT - 1),
                )
            nc.scalar.copy(v_sb[:, mt, :], ps[:])

    # ---- pipelined emission: projections feed per-head attention
    proj_qk("q", 0, xt_hi_dma=(xt_lo_b, xt_hi_a, xt_hi_b, xt_t))
    proj_qk("k", 0)
    mask_prefetch()
    mask_prefetch()
    proj_v()
    for h in range(HPC):
        qk_phase(h)
        if h + 1 < HPC:
            proj_qk("q", h + 1)
            proj_qk("k", h + 1)
        av_phase(h)

    # ---- out projection partial (copy psum->sbuf bf16, DMA to DRAM)
    wo_tiles = {}

    def wo_load(n):
        if n >= D // 512:
            return
        t = wstream.tile([P, HPC, 512], bf16, tag="wo")
        nc.sync.dma_start(t[:], wo_t[:, :, n * 512 : (n + 1) * 512])
        wo_tiles[n] = t

    for n in range(D // 512):
        if n == 0:
            wo_load(0)
        wo_load(n + 1)
        wo_sb = wo_tiles.pop(n)
        for mt in range(NMI):
            ps = ppsum.tile([P, 512], f32, tag="pp")
            for kt in range(HPC):
                nc.tensor.matmul(
                    ps[:],
                    attnT[:, kt, mt * P : (mt + 1) * P],
                    wo_sb[:, kt, :],
                    start=(kt == 0),
                    stop=(kt == HPC - 1),
                )
            ot = outp.tile([P, 512], bf16, tag="ot")
            if mt % 2 == 0:
                nc.scalar.copy(ot[:], ps[:])
            else:
                nc.vector.tensor_scalar_mul(ot[:], ps[:], 1.0)
            nc.sync.dma_start(
                out[mt * P : (mt + 1) * P, n * 512 : (n + 1) * 512], ot[:]
            )


def _rope_tables():
    """cos/sin tables in transposed-projection layout [128 dims, TOK],
    with rotate-half sign folded into sin and 1/sqrt(dh) folded into the
    q tables."""
    inv_freq = 1.0 / (
        ROPE_BASE ** (np.arange(0, DH, 2, dtype=np.float32) / DH)
    )  # [64]
    t = np.arange(S, dtype=np.float32)
    freqs = np.outer(t, inv_freq)  # [S, 64]
    cos = np.cos(freqs)
    sin = np.sin(freqs)
    cos2 = np.empty((P, TOK), np.float32)
    sin2 = np.empty((P, TOK), np.float32)
    cos2[0:64] = cos.T
    cos2[64:128] = cos.T
    sin2[0:64] = -sin.T
    sin2[64:128] = sin.T
    scale = 1.0 / math.sqrt(DH)
    return cos2 * scale, sin2 * scale, cos2, sin2


def make_in_maps(x, prophetic_mask, policy_mask, memory_weights, Wq, Wk, Wv, Wo):
    from concurrent.futures import ThreadPoolExecutor

    bf16 = ml_dtypes.bfloat16
    cosq, sinq, cosk, sink = _rope_tables()
    ident = np.eye(P, dtype=np.float32).astype(bf16)
    ppx, ccx = np.meshgrid(np.arange(P), np.arange(P), indexing="ij")
    cneg = np.where(ccx <= ppx, 0.0, -1.0e30).astype(bf16)
    wq_b = np.ascontiguousarray(Wq).astype(bf16)
    wk_b = np.ascontiguousarray(Wk).astype(bf16)
    wv_b = np.ascontiguousarray(Wv).astype(bf16)
    wo_b = np.ascontiguousarray(Wo).astype(bf16)
    # fold the governance scales into the host-side bf16 cast; the mem
    # weights use the chord fit log(1+GS*m) ~= log1p(GS)*m
    c_mem = np.float32(np.log1p(GS))

    def shard(c):
        b = c // 4
        g = c % 4
        cols = slice(DLOC * g, DLOC * (g + 1))
        hsl = slice(HPC * g, HPC * (g + 1))
        return {
            "xt": np.ascontiguousarray(x[b].T).astype(bf16),
            "wq": np.ascontiguousarray(wq_b[:, cols]),
            "wk": np.ascontiguousarray(wk_b[:, cols]),
            "wv": np.ascontiguousarray(wv_b[:, cols]),
            "wo": np.ascontiguousarray(wo_b[cols, :]),
            "pm": (prophetic_mask[b, hsl] * np.float32(GS * 0.5)).astype(bf16),
            "pol": (policy_mask[b, hsl] * np.float32(GS)).astype(bf16),
            "mem": (memory_weights[b, hsl] * c_mem).astype(bf16),
            "cosq": cosq,
            "sinq": sinq,
            "cosk": cosk,
            "sink": sink,
            "ident": ident,
            "cneg": cneg,
        }

    with ThreadPoolExecutor(8) as ex:
        in_maps = list(ex.map(shard, range(NCORES)))
    return in_maps


def kernel(x, prophetic_mask, policy_mask, memory_weights, Wq, Wk, Wv, Wo, bo):
    from concourse.bass_utils import run_bass_kernel_spmd

    if "nc" not in _CACHE:
        _CACHE["nc"] = build_nc()
    nc = _CACHE["nc"]
    in_maps = make_in_maps(
        x, prophetic_mask, policy_mask, memory_weights, Wq, Wk, Wv, Wo
    )
    res = run_bass_kernel_spmd(nc, in_maps, list(range(NCORES)))
    out = np.zeros((B, S, D), np.float32)
    for c in range(NCORES):
        out[c // 4] += res.results[c]["out"].astype(np.float32)
    out += np.asarray(bo, np.float32)[None, None, :]
    return out
